# revision 1
# baseline (speedup 1.0000x reference)
"""Embedding lookup (gather + scale) on 8 TRN2 NeuronCores.

Strategy: data-parallel over tokens. The [50257, 1024] f32 table is
replicated to every core's DRAM; the 8*2048 = 16384 tokens are split into 8
chunks of 2048. Each core gathers its 2048 rows with indirect DMA
(128 rows per gather), scales by sqrt(1024) = 32 on the vector engine, and
stores its [2048, 1024] f32 slice. No collectives.
"""

import math

import numpy as np

D_VOCAB = 50257
D_MODEL = 1024
N_CORES = 8
TOK_PER_CORE = 2048
P = 128
N_TILES = TOK_PER_CORE // P  # 16
SCALE = math.sqrt(D_MODEL)  # 32.0

_progs = {}


def _build_program(reps=1, chunk_cols=1, bufs=14):
    """Build the per-core Bass program. reps>1 repeats the whole pipeline
    back-to-back inside one NEFF (benchmarking only — output is idempotent).
    chunk_cols: tokens-per-partition per gather chunk (chunk = 128*chunk_cols
    rows)."""
    import concourse.bacc as bacc
    import concourse.mybir as mybir
    import concourse.tile as tile
    from concourse import bass

    nc = bacc.Bacc("TRN2", debug=False, num_devices=N_CORES)
    tokens = nc.dram_tensor(
        "tokens", [TOK_PER_CORE], mybir.dt.int32, kind="ExternalInput"
    ).ap()
    w = nc.dram_tensor(
        "w", [D_VOCAB, D_MODEL], mybir.dt.float32, kind="ExternalInput"
    ).ap()
    out = nc.dram_tensor(
        "out", [TOK_PER_CORE, D_MODEL], mybir.dt.float32, kind="ExternalOutput"
    ).ap()

    # The host uploads tokens PRE-PERMUTED: tokens_in[p*16 + j] =
    # original_tokens[j*128 + p]. The [128, 16] idx load is then one
    # contiguous 64B-per-partition DMA, gather j's column j holds the indices
    # for output rows j*128..(j+1)*128, and every store is a fully contiguous
    # 512KB block.
    with tile.TileContext(nc) as tc:
        with (
            tc.tile_pool(name="idx", bufs=1) as idx_pool,
            tc.tile_pool(name="emb", bufs=bufs) as emb_pool,
        ):
            idx_tile = idx_pool.tile([P, N_TILES], mybir.dt.int32)
            nc.sync.dma_start(
                out=idx_tile[:], in_=tokens.rearrange("(p j) -> p j", p=P)
            )
            H = D_MODEL // 2
            for r in range(reps):
                for j in range(N_TILES):
                    emb = emb_pool.tile([P, D_MODEL], mybir.dt.float32)
                    nc.gpsimd.indirect_dma_start(
                        out=emb[:],
                        out_offset=None,
                        in_=w[:],
                        in_offset=bass.IndirectOffsetOnAxis(
                            ap=idx_tile[:, j : j + 1], axis=0
                        ),
                    )
                    # alternate engines so neither scale nor store DGE binds
                    if j % 2 == 0:
                        nc.vector.tensor_scalar_mul(emb[:], emb[:], SCALE)
                    else:
                        nc.scalar.mul(emb[:], emb[:], SCALE)
                    store_eng = nc.sync if j % 2 == 0 else nc.scalar
                    store_eng.dma_start(
                        out=out[j * P : (j + 1) * P, :], in_=emb[:]
                    )

    nc.compile()
    return nc


def _get_program(reps=1, chunk_cols=1, bufs=14):
    key = (reps, chunk_cols, bufs)
    if key not in _progs:
        _progs[key] = _build_program(reps, chunk_cols, bufs)
    return _progs[key]


def _run(tokens, W_E, trace=False):
    from concourse.bass_utils import run_bass_kernel_spmd

    tokens = np.ascontiguousarray(np.asarray(tokens).astype(np.int32))
    W_E = np.ascontiguousarray(np.asarray(W_E, dtype=np.float32))
    assert tokens.shape == (N_CORES * TOK_PER_CORE // TOK_PER_CORE, TOK_PER_CORE) or (
        tokens.size == N_CORES * TOK_PER_CORE
    )
    flat = tokens.reshape(-1)

    nc = _get_program()
    in_maps = []
    for c in range(N_CORES):
        chunk = flat[c * TOK_PER_CORE : (c + 1) * TOK_PER_CORE]
        # device expects tokens_in[p*16 + j] = chunk[j*128 + p]
        permuted = np.ascontiguousarray(chunk.reshape(N_TILES, P).T.reshape(-1))
        in_maps.append({"tokens": permuted, "w": W_E})
    res = run_bass_kernel_spmd(
        nc, in_maps, core_ids=list(range(N_CORES)), trace=trace
    )
    out = np.stack([res.results[c]["out"] for c in range(N_CORES)], axis=0)
    return out.reshape(N_CORES, TOK_PER_CORE, D_MODEL), res


def kernel(tokens, W_E):
    out, _ = _run(tokens, W_E, trace=False)
    return out



# revision 3
# speedup vs baseline: 1.1700x; 1.1700x over previous
"""Embedding lookup (gather + scale) on 8 TRN2 NeuronCores.

Strategy: data-parallel over tokens. The [50257, 1024] table is converted to
bf16 on the host (rel err ~3e-3, well under the 2e-2 gate) and replicated to
every core's DRAM; the 8*2048 = 16384 tokens are split into 8 chunks of 2048.
Each core gathers its 2048 rows with indirect DMA (bf16, halving HBM read
traffic vs f32), upcasts+scales by sqrt(1024) = 32 on the scalar (ACT)
engine — the DVE cross-dtype tensor_scalar path miscompiles on HW — and
stores its [2048, 1024] f32 slice via HWDGE. No collectives.

HW-validated constraints (sim accepts both, HW does not):
- indirect_dma_start offset AP must be a single column ([128, 1]);
  multi-column offset APs crash the exec unit.
- vector.tensor_scalar_mul with in=bf16/out=f32 produces garbage; the
  ACT activation (Copy w/ scale) path converts correctly.
"""

import math

import numpy as np

D_VOCAB = 50257
D_MODEL = 1024
N_CORES = 8
TOK_PER_CORE = 2048
P = 128
N_TILES = TOK_PER_CORE // P  # 16
SCALE = math.sqrt(D_MODEL)  # 32.0

_progs = {}


def _build_program(bufs=6):
    import concourse.bacc as bacc
    import concourse.mybir as mybir
    import concourse.tile as tile
    from concourse import bass

    nc = bacc.Bacc("TRN2", debug=False, num_devices=N_CORES)
    tokens = nc.dram_tensor(
        "tokens", [TOK_PER_CORE], mybir.dt.int32, kind="ExternalInput"
    ).ap()
    w = nc.dram_tensor(
        "w", [D_VOCAB, D_MODEL], mybir.dt.bfloat16, kind="ExternalInput"
    ).ap()
    out = nc.dram_tensor(
        "out", [TOK_PER_CORE, D_MODEL], mybir.dt.float32, kind="ExternalOutput"
    ).ap()

    # The host uploads tokens PRE-PERMUTED: tokens_in[p*16 + j] =
    # original_tokens[j*128 + p]. The [128, 16] idx load is then one
    # contiguous 64B-per-partition DMA, gather j's column j holds the indices
    # for output rows j*128..(j+1)*128, and every store is a fully contiguous
    # 512KB block.
    with tile.TileContext(nc) as tc:
        with (
            tc.tile_pool(name="idx", bufs=1) as idx_pool,
            tc.tile_pool(name="embb", bufs=bufs) as bf_pool,
            tc.tile_pool(name="embf", bufs=bufs) as f_pool,
        ):
            idx_tile = idx_pool.tile([P, N_TILES], mybir.dt.int32)
            nc.sync.dma_start(
                out=idx_tile[:], in_=tokens.rearrange("(p j) -> p j", p=P)
            )
            for j in range(N_TILES):
                emb_bf = bf_pool.tile([P, D_MODEL], mybir.dt.bfloat16)
                nc.gpsimd.indirect_dma_start(
                    out=emb_bf[:],
                    out_offset=None,
                    in_=w[:],
                    in_offset=bass.IndirectOffsetOnAxis(
                        ap=idx_tile[:, j : j + 1], axis=0
                    ),
                )
                emb_f = f_pool.tile([P, D_MODEL], mybir.dt.float32)
                nc.scalar.mul(emb_f[:], emb_bf[:], SCALE)
                nc.sync.dma_start(out=out[j * P : (j + 1) * P, :], in_=emb_f[:])

    nc.compile()
    return nc


def _get_program(bufs=6):
    key = (bufs,)
    if key not in _progs:
        _progs[key] = _build_program(bufs)
    return _progs[key]


def _to_bf16(a):
    """f32 -> bf16 with round-to-nearest-even, returned as ml_dtypes.bfloat16."""
    import ml_dtypes

    return np.asarray(a, dtype=np.float32).astype(ml_dtypes.bfloat16)


def _run(tokens, W_E, trace=False, core_ids=None, **prog_kwargs):
    from concourse.bass_utils import run_bass_kernel_spmd

    tokens = np.ascontiguousarray(np.asarray(tokens).astype(np.int32))
    assert tokens.size == N_CORES * TOK_PER_CORE
    flat = tokens.reshape(-1)
    w_bf = np.ascontiguousarray(_to_bf16(W_E))

    if core_ids is None:
        core_ids = list(range(N_CORES))
    nc = _get_program(**prog_kwargs)
    in_maps = []
    for c in core_ids:
        chunk = flat[c * TOK_PER_CORE : (c + 1) * TOK_PER_CORE]
        # device expects tokens_in[p*16 + j] = chunk[j*128 + p]
        permuted = np.ascontiguousarray(chunk.reshape(N_TILES, P).T.reshape(-1))
        in_maps.append({"tokens": permuted, "w": w_bf})
    res = run_bass_kernel_spmd(nc, in_maps, core_ids=core_ids, trace=trace)
    out = np.stack([res.results[i]["out"] for i in range(len(core_ids))], axis=0)
    return out.reshape(len(core_ids), TOK_PER_CORE, D_MODEL), res


def kernel(tokens, W_E):
    out, _ = _run(tokens, W_E, trace=False)
    return out


# revision 6
# speedup vs baseline: 1.4283x; 1.2208x over previous
"""Embedding lookup (gather + scale) on 8 TRN2 NeuronCores.

Strategy: data-parallel over tokens. The [50257, 1024] table is converted to
bf16 on the host (rel err ~3e-3, well under the 2e-2 gate) and replicated to
every core's DRAM; the 8*2048 = 16384 tokens are split into 8 chunks of 2048.
Each core gathers its 2048 rows with indirect DMA (bf16, halving HBM read
traffic vs f32), scales by sqrt(1024) = 32 in-place on the vector engine
(exact in bf16: pure exponent shift), and stores its [2048, 1024] slice as
bf16 (halving HBM write traffic). The host widens bf16 -> f32, which is
numerically exact, so the returned output is bit-identical to an on-device
upcast. No collectives.

HW-validated constraints (sim accepts these, HW does not):
- indirect_dma_start offset AP must be a single column ([128, 1]);
  multi-column offset APs crash the exec unit.
- indirect_dma_start dest must lower to a plain 2D AP; a [128, 1, 1024]
  slice of a 3D tile lands in the wrong place. Flat column slices of a
  2D tile (emb[:, i*1024:(i+1)*1024]) work.
- Cross-dtype DVE ops (bf16 in -> f32 out) produce garbage; same-dtype
  in-place ops are fine.
"""

import math

import numpy as np

D_VOCAB = 50257
D_MODEL = 1024
N_CORES = 8
TOK_PER_CORE = 2048
P = 128
N_TILES = TOK_PER_CORE // P  # 16
SCALE = math.sqrt(D_MODEL)  # 32.0

_progs = {}


def _build_program(pair=2, bufs=8):
    import concourse.bacc as bacc
    import concourse.mybir as mybir
    import concourse.tile as tile
    from concourse import bass

    nc = bacc.Bacc("TRN2", debug=False, num_devices=N_CORES)
    tokens = nc.dram_tensor(
        "tokens", [TOK_PER_CORE], mybir.dt.int32, kind="ExternalInput"
    ).ap()
    w = nc.dram_tensor(
        "w", [D_VOCAB, D_MODEL], mybir.dt.bfloat16, kind="ExternalInput"
    ).ap()
    out = nc.dram_tensor(
        "out", [TOK_PER_CORE, D_MODEL], mybir.dt.bfloat16, kind="ExternalOutput"
    ).ap()

    n_groups = N_TILES // pair
    # The host uploads tokens PRE-PERMUTED: tokens_in[p*16 + j] =
    # original_tokens[j*128 + p]. The [128, 16] idx load is then one
    # contiguous 64B-per-partition DMA; gather j's column j holds the indices
    # for output rows j*128..(j+1)*128. Gathers land pairwise in one tile so
    # one DVE scale and one 512KB store cover two gathers.
    with tile.TileContext(nc) as tc:
        with (
            tc.tile_pool(name="idx", bufs=1) as idx_pool,
            tc.tile_pool(name="emb", bufs=bufs) as emb_pool,
        ):
            idx_tile = idx_pool.tile([P, N_TILES], mybir.dt.int32)
            nc.sync.dma_start(
                out=idx_tile[:], in_=tokens.rearrange("(p j) -> p j", p=P)
            )
            for g in range(n_groups):
                emb = emb_pool.tile([P, pair * D_MODEL], mybir.dt.bfloat16)
                for i in range(pair):
                    j = g * pair + i
                    nc.gpsimd.indirect_dma_start(
                        out=emb[:, i * D_MODEL : (i + 1) * D_MODEL],
                        out_offset=None,
                        in_=w[:],
                        in_offset=bass.IndirectOffsetOnAxis(
                            ap=idx_tile[:, j : j + 1], axis=0
                        ),
                    )
                nc.vector.tensor_scalar_mul(emb[:], emb[:], SCALE)
                # SBUF (p, i) holds token row g*pair*128 + i*128 + p
                dram_view = out[g * pair * P : (g + 1) * pair * P, :].rearrange(
                    "(i p) m -> p i m", p=P
                )
                sbuf_view = emb[:].rearrange("p (i m) -> p i m", m=D_MODEL)
                store_eng = nc.sync if g % 2 == 0 else nc.scalar
                store_eng.dma_start(out=dram_view, in_=sbuf_view)

    nc.compile()
    return nc


def _get_program(pair=2, bufs=8):
    key = (pair, bufs)
    if key not in _progs:
        _progs[key] = _build_program(pair, bufs)
    return _progs[key]


def _to_bf16(a):
    """f32 -> bf16 with round-to-nearest-even, returned as ml_dtypes.bfloat16."""
    import ml_dtypes

    return np.asarray(a, dtype=np.float32).astype(ml_dtypes.bfloat16)


def _run(tokens, W_E, trace=False, core_ids=None, **prog_kwargs):
    from concourse.bass_utils import run_bass_kernel_spmd

    tokens = np.ascontiguousarray(np.asarray(tokens).astype(np.int32))
    assert tokens.size == N_CORES * TOK_PER_CORE
    flat = tokens.reshape(-1)
    w_bf = np.ascontiguousarray(_to_bf16(W_E))

    if core_ids is None:
        core_ids = list(range(N_CORES))
    nc = _get_program(**prog_kwargs)
    in_maps = []
    for c in core_ids:
        chunk = flat[c * TOK_PER_CORE : (c + 1) * TOK_PER_CORE]
        # device expects tokens_in[p*16 + j] = chunk[j*128 + p]
        permuted = np.ascontiguousarray(chunk.reshape(N_TILES, P).T.reshape(-1))
        in_maps.append({"tokens": permuted, "w": w_bf})
    res = run_bass_kernel_spmd(nc, in_maps, core_ids=core_ids, trace=trace)
    # device output is bf16; widening to f32 is exact (pure mantissa pad)
    out = np.stack(
        [np.asarray(res.results[i]["out"], dtype=np.float32) for i in range(len(core_ids))],
        axis=0,
    )
    return out.reshape(len(core_ids), TOK_PER_CORE, D_MODEL), res


def kernel(tokens, W_E):
    out, _ = _run(tokens, W_E, trace=False)
    return out
